# revision 1
# baseline (speedup 1.0000x reference)
"""ChamferLoss Trainium2 kernel.

Strategy (per core, data-parallel over batch: 16 batches / 8 cores = 2 each):
  pdist[b,i,j] = ||x_i||^2 + ||y_j||^2 - 2 x_i.y_j   (first 3 channels)
  loss = mean_bj(min_i pdist) + mean_bi(min_j pdist)

m = -pdist comes from a single K=13 bf16 augmented matmul (hi/lo split gives
fp32-class accuracy at bf16 PE speed):
  x-side rows: [xh(3), xh(3), xl(3), -rxh, -rxl, -1, -1]
  y-side rows: [Yh(3), Yl(3), Yh(3),  1,    1,  Ryh, Ryl],  Y = 2y, Ry=||y||^2
min -> max flip: rowmax via tensor_scalar+accum, colmax via tensor_tensor max.

CRITICAL environment fact (measured): on this axon execution path every
cross-engine semaphore dependency costs ~30-70 us, while back-to-back work on
one engine runs at full speed.  The kernel is therefore structured to minimise
cross-engine edges, not engine-seconds:
  - one full-PSUM fill per (batch,row-tile): 8 matmuls -> [128,4096] f32,
    then exactly one PE->DVE handoff and one DVE->PE handback (64 fills/core)
  - no ACT cast; DVE reduces straight from PSUM in fp32
  - all prep arithmetic on DVE only; PE transposes build the channel-major
    operands on-chip (no strided DRAM round-trips - those are ms-slow)
  - output is per-partition partial sums [128,4]; the host does the final
    128-way gather-sum (pure unsharding arithmetic)
"""

from contextlib import ExitStack

import numpy as np

import concourse.bass as bass
import concourse.bacc as bacc
import concourse.tile as tile
from concourse import bass_isa, mybir
from concourse.bass_utils import run_bass_kernel_spmd
from concourse.masks import make_identity

F32 = mybir.dt.float32
BF16 = mybir.dt.bfloat16
AX = mybir.AxisListType
OP = mybir.AluOpType

NEG_BIG = -3.0e38

B_FULL = 16
N_FULL = 4096
C_FULL = 6
N_CORES = 8


def build_nc(b_loc=2, n=4096, c_in=6, num_devices=8, reps=1):
    """Per-core program. Inputs x,y: [b_loc, n, c_in] f32; output "partial"
    [128, 2*b_loc] f32 per-partition partial sums of rowmax/colmax of -pdist."""
    NP = 128
    NQ = n // NP                  # row-tiles per batch (32)
    TH = NQ                       # transposes per prep psum fill

    nc = bacc.Bacc(
        "TRN2",
        target_bir_lowering=False,
        debug=False,
        enable_asserts=False,
        num_devices=num_devices,
    )

    x_d = nc.declare_dram_parameter("x", [b_loc, n, c_in], F32, isOutput=False).ap()
    y_d = nc.declare_dram_parameter("y", [b_loc, n, c_in], F32, isOutput=False).ap()
    out_d = nc.declare_dram_parameter(
        "partial", [NP, 2 * b_loc], F32, isOutput=True
    ).ap()

    with tile.TileContext(nc) as tc, ExitStack() as ctx:
        prep = ctx.enter_context(tc.tile_pool(name="prep", bufs=2))
        singles = ctx.enter_context(tc.tile_pool(name="singles", bufs=1))
        psum_pool = ctx.enter_context(tc.tile_pool(name="psum", bufs=1, space="PSUM"))
        smalls = ctx.enter_context(tc.tile_pool(name="smalls", bufs=2))

        ident = singles.tile([NP, NP], BF16, tag="ident", name="ident")
        make_identity(nc, ident)
        ident32 = singles.tile([NP, NP], F32, tag="ident32", name="ident32")
        make_identity(nc, ident32)

        def emit_body():
            chx = [singles.tile([13, n], BF16, tag=f"chx{b}", name=f"chx{b}")
                   for b in range(b_loc)]
            chy = [singles.tile([13, n], BF16, tag=f"chy{b}", name=f"chy{b}")
                   for b in range(b_loc)]

            # ---- prep: aug point-major (DVE only), PE-transpose, DVE evac
            for b in range(b_loc):
                for side in ("x", "y"):
                    src = x_d if side == "x" else y_d
                    xin = prep.tile([NP, NQ, c_in], F32, tag="xin")
                    nc.sync.dma_start(
                        out=xin, in_=src[b].rearrange("(p q) c -> p q c", p=NP)
                    )
                    aug = prep.tile([NP, NQ, 13], BF16, tag="aug")
                    sq = prep.tile([NP, NQ, 3], F32, tag="sq")
                    rt = prep.tile([NP, NQ, 1], F32, tag="rt")
                    ch = xin[:, :, 0:3]
                    nc.vector.tensor_mul(sq, ch, ch)
                    nc.vector.tensor_reduce(rt, sq, axis=AX.X, op=OP.add)
                    if side == "x":
                        # [xh xh xl | -rxh -rxl | -1 -1]
                        nc.vector.tensor_copy(aug[:, :, 0:3], ch)
                        nc.vector.tensor_copy(aug[:, :, 3:6], aug[:, :, 0:3])
                        nc.vector.tensor_sub(aug[:, :, 6:9], ch, aug[:, :, 0:3])
                        nc.vector.tensor_scalar_mul(aug[:, :, 9:10], rt, -1.0)
                        nc.vector.scalar_tensor_tensor(
                            aug[:, :, 10:11], rt, -1.0, aug[:, :, 9:10],
                            OP.mult, OP.subtract,
                        )
                        nc.vector.memset(aug[:, :, 11:13], -1.0)
                    else:
                        # [Yh Yl Yh | 1 1 | ryh ryl],  Y = 2y
                        nc.vector.tensor_scalar_mul(aug[:, :, 0:3], ch, 2.0)
                        nc.vector.scalar_tensor_tensor(
                            aug[:, :, 3:6], ch, 2.0, aug[:, :, 0:3],
                            OP.mult, OP.subtract,
                        )
                        nc.vector.tensor_copy(aug[:, :, 6:9], aug[:, :, 0:3])
                        nc.vector.memset(aug[:, :, 9:11], 1.0)
                        nc.vector.tensor_copy(aug[:, :, 11:12], rt)
                        nc.vector.tensor_sub(aug[:, :, 12:13], rt, aug[:, :, 11:12])

                    # one PSUM fill: 32 transposes, then one DVE evacuation
                    pt = psum_pool.tile([NP, n], BF16, tag="ps")
                    for q in range(TH):
                        nc.tensor.transpose(
                            pt[0:13, q * NP : (q + 1) * NP], aug[:, q, :], ident
                        )
                    dst = chx[b] if side == "x" else chy[b]
                    nc.vector.tensor_copy(dst, pt[0:13, :])

            # ---- accumulators (all DVE-resident) ----
            colacc = [singles.tile([NP, n], F32, tag=f"colacc{b}", name=f"colacc{b}")
                      for b in range(b_loc)]
            for b in range(b_loc):
                nc.vector.memset(colacc[b], NEG_BIG)
            rowpart = [singles.tile([NP, NQ], F32, tag=f"rowpart{b}",
                                    name=f"rowpart{b}") for b in range(b_loc)]
            junk = singles.tile([NP, n], F32, tag="junk", name="junk")

            # ---- main: 64 full-PSUM fills, one PE->DVE->PE round-trip each;
            # ONE psum tile reused across fills (no per-fill TileRelease)
            ps = psum_pool.tile([NP, n], F32, tag="ps", name="ps_main")
            for b in range(b_loc):
                for r in range(NQ):
                    lhsT = chx[b][:, r * NP : (r + 1) * NP]
                    for s in range(n // 512):
                        nc.tensor.matmul(
                            ps[:, s * 512 : (s + 1) * 512],
                            lhsT=lhsT,
                            rhs=chy[b][:, s * 512 : (s + 1) * 512],
                            start=True,
                            stop=True,
                        )
                    nc.vector.tensor_scalar(
                        out=junk,
                        in0=ps,
                        scalar1=NEG_BIG,
                        scalar2=None,
                        op0=OP.max,
                        op1=OP.max,
                        accum_out=rowpart[b][:, r : r + 1],
                    )
                    nc.vector.tensor_tensor(colacc[b], colacc[b], ps, op=OP.max)

            # ---- finals ----
            sums = singles.tile([NP, 2 * b_loc], F32, tag="sums", name="sums")
            for b in range(b_loc):
                # row side: max over the two half-row partials, then sum
                nc.vector.tensor_reduce(sums[:, b : b + 1], rowpart[b],
                                        axis=AX.X, op=OP.add)
                # col side: transpose colacc, rowmax-reduce, sum
                cmax = smalls.tile([NP, NQ], F32, tag="cmax")
                for t in range(NQ):
                    nc.tensor.transpose(
                        ps[:, t * NP : (t + 1) * NP],
                        colacc[b][:, t * NP : (t + 1) * NP],
                        ident32,
                    )
                nc.vector.tensor_reduce(
                    cmax, ps.rearrange("p (t v) -> p t v", t=NQ),
                    axis=AX.X, op=OP.max,
                )
                nc.vector.tensor_reduce(sums[:, b_loc + b : b_loc + b + 1], cmax,
                                        axis=AX.X, op=OP.add)
            nc.sync.dma_start(out=out_d, in_=sums)

        for _ in range(reps):
            emit_body()

    nc.compile()
    return nc


_CACHE = {}


def _get_nc():
    if "nc" not in _CACHE:
        _CACHE["nc"] = build_nc(
            b_loc=B_FULL // N_CORES, n=N_FULL, c_in=C_FULL, num_devices=N_CORES
        )
    return _CACHE["nc"]


def kernel(x: np.ndarray, y: np.ndarray) -> np.ndarray:
    x = np.ascontiguousarray(np.asarray(x, dtype=np.float32))
    y = np.ascontiguousarray(np.asarray(y, dtype=np.float32))
    assert x.shape == (B_FULL, N_FULL, C_FULL), x.shape
    nc = _get_nc()
    bl = B_FULL // N_CORES
    in_maps = [
        {
            "x": np.ascontiguousarray(x[i * bl : (i + 1) * bl]),
            "y": np.ascontiguousarray(y[i * bl : (i + 1) * bl]),
        }
        for i in range(N_CORES)
    ]
    res = run_bass_kernel_spmd(nc, in_maps, list(range(N_CORES)))
    total = sum(float(r["partial"].astype(np.float64).sum()) for r in res.results)
    loss = -total / float(B_FULL * N_FULL)
    return np.float32(loss)



# revision 3
# speedup vs baseline: 19.0065x; 19.0065x over previous
"""ChamferLoss Trainium2 kernel (v2 — hardware-loop structured).

Data-parallel over batch: 16 batches / 8 cores = 2 each.
  m[b,i,j] = -pdist = 2 x_i.y_j - ||x_i||^2 - ||y_j||^2   (first 3 channels)
  loss = -( mean_bi max_j m + mean_bj max_i m )

The cross term comes from a single K=13 bf16 augmented matmul (hi/lo split
gives fp32-class accuracy at bf16 PE speed):
  x-side rows: [xh(3), xh(3), xl(3), -rxh, -rxl, -1, -1]
  y-side rows: [Yh(3), Yl(3), Yh(3),  1,    1,  Ryh, Ryl],  Y = 2y, Ry=||y||^2
The augmented operands are built ON THE HOST (exact fp32->bf16 hi/lo split,
channel-major) and shipped as one contiguous [13, 4, 4096] bf16 tensor, so
the device does zero prep work (no PE transposes, no DVE augmentation).

Measured cost model for this axon/PJRT execution path (see probes.py):
  - every *static* instruction costs ~17-90 us (matmul ~46 us, DVE ~43 us,
    DMA ~34 us, ACT ~89 us) regardless of its size;
  - engines do NOT overlap;
  - instructions inside a tc.For_i hardware loop pay the tax ONCE (static),
    plus ~15-25 us of back-edge sync per iteration.
The kernel is therefore one 32-iteration hardware loop whose body handles
one 128-row tile of BOTH local batches: 16 matmuls + 4 DVE ops static.
Finals use a GPSIMD partition-axis reduce (no transposes).  Output is a
small [128, 4] partial tile; the host does the final gather-sum.
"""

from contextlib import ExitStack

import numpy as np

import concourse.bass as bass
import concourse.bacc as bacc
import concourse.tile as tile
from concourse import mybir
from concourse.bass import ds
from concourse.bass_utils import run_bass_kernel_spmd

F32 = mybir.dt.float32
BF16 = mybir.dt.bfloat16
AX = mybir.AxisListType
OP = mybir.AluOpType

NEG_BIG = -3.0e38

B_FULL = 16
N_FULL = 4096
C_FULL = 6
N_CORES = 8
KAUG = 13


def build_nc(b_loc=2, n=4096, c_in=6, num_devices=8, reps=1):
    """Per-core program. Input aug: [13, 2*b_loc, n] bf16 (host-prepped
    augmented operands: cols 0..b_loc-1 = x-side, b_loc..2*b_loc-1 = y-side);
    output "partial" [128, 2*b_loc] f32:
      partial[:, b]            = per-partition sums of rowmax (x side)
      partial[0, b_loc + b]    = total colmax sum (y side), rest zeros.
    """
    NP = 128
    NQ = n // NP                  # row-tiles per batch (32)
    NS = n // 512                 # 512-wide matmul slabs (8)

    nc = bacc.Bacc(
        "TRN2",
        target_bir_lowering=False,
        debug=False,
        enable_asserts=False,
        num_devices=num_devices,
    )

    aug_d = nc.declare_dram_parameter(
        "aug", [KAUG, 2 * b_loc, n], BF16, isOutput=False
    ).ap()
    out_d = nc.declare_dram_parameter(
        "partial", [NP, 2 * b_loc], F32, isOutput=True
    ).ap()

    with tile.TileContext(nc) as tc, ExitStack() as ctx:
        singles = ctx.enter_context(tc.tile_pool(name="singles", bufs=1))
        psum_pool = ctx.enter_context(tc.tile_pool(name="psum", bufs=1, space="PSUM"))

        def emit_body():
            aug_s = singles.tile([KAUG, 2 * b_loc, n], BF16, tag="aug", name="aug_s")
            nc.sync.dma_start(out=aug_s, in_=aug_d)

            colacc = [singles.tile([NP, n], F32, tag=f"colacc{b}",
                                   name=f"colacc{b}") for b in range(b_loc)]
            rowpart = [singles.tile([NP, NQ], F32, tag=f"rowpart{b}",
                                    name=f"rowpart{b}") for b in range(b_loc)]
            sums = singles.tile([NP, 2 * b_loc], F32, tag="sums", name="sums")
            cm = singles.tile([1, n], F32, tag="cm", name="cm")
            for b in range(b_loc):
                nc.vector.memset(colacc[b], NEG_BIG)
            nc.vector.memset(sums, 0.0)

            ps = psum_pool.tile([NP, n], F32, tag="ps", name="ps_main")
            # walrus can't register-offset ldweights, so stage the moving
            # row-tile of weights through a fixed-address tile per batch.
            wt = [singles.tile([KAUG, NP], BF16, tag=f"wt{b}", name=f"wt{b}")
                  for b in range(b_loc)]

            with tc.For_i(0, NQ, 1) as ri:
                for b in range(b_loc):
                    nc.vector.tensor_copy(wt[b], aug_s[:, b, ds(ri * NP, NP)])
                    for s in range(NS):
                        nc.tensor.matmul(
                            ps[:, s * 512:(s + 1) * 512],
                            lhsT=wt[b],
                            rhs=aug_s[:, b_loc + b, s * 512:(s + 1) * 512],
                            start=True,
                            stop=True,
                        )
                    nc.vector.tensor_reduce(
                        rowpart[b][:, ds(ri, 1)], ps, axis=AX.X, op=OP.max
                    )
                    nc.vector.tensor_tensor(colacc[b], colacc[b], ps, op=OP.max)

            for b in range(b_loc):
                nc.vector.tensor_reduce(
                    sums[:, b:b + 1], rowpart[b], axis=AX.X, op=OP.add
                )
                nc.gpsimd.tensor_reduce(cm, colacc[b], axis=AX.C, op=OP.max)
                nc.vector.tensor_reduce(
                    sums[0:1, b_loc + b:b_loc + b + 1], cm, axis=AX.X, op=OP.add
                )
            nc.sync.dma_start(out=out_d, in_=sums)

        for _ in range(reps):
            emit_body()

    nc.compile()
    return nc


def _host_aug(x: np.ndarray, y: np.ndarray) -> np.ndarray:
    """Build the augmented [13, 2*b, n] bf16 operand block for one core.

    x, y: [b, n, 6] f32.  Coordinate channels are the first 3.
    """
    import ml_dtypes

    b, n, _ = x.shape
    xc = np.ascontiguousarray(x[:, :, :3]).astype(np.float32)   # [b, n, 3]
    yc = np.ascontiguousarray(y[:, :, :3]).astype(np.float32)

    def split(v):
        hi = v.astype(ml_dtypes.bfloat16).astype(np.float32)
        lo = (v - hi).astype(ml_dtypes.bfloat16).astype(np.float32)
        return hi, lo

    xh, xl = split(xc)                                  # [b, n, 3]
    rx = np.sum(xc * xc, axis=-1)                       # [b, n]
    rxh, rxl = split(rx)

    Y = 2.0 * yc
    Yh, Yl = split(Y)
    ry = np.sum(yc * yc, axis=-1)
    ryh, ryl = split(ry)

    ones = np.ones_like(rx)

    # x-side rows (K=13): [xh(3), xh(3), xl(3), -rxh, -rxl, -1, -1]
    ax = np.concatenate(
        [xh, xh, xl, -rxh[..., None], -rxl[..., None],
         -ones[..., None], -ones[..., None]], axis=-1)   # [b, n, 13]
    # y-side rows: [Yh(3), Yl(3), Yh(3), 1, 1, ryh, ryl]
    ay = np.concatenate(
        [Yh, Yl, Yh, ones[..., None], ones[..., None],
         ryh[..., None], ryl[..., None]], axis=-1)       # [b, n, 13]

    # -> [13, 2b, n] channel-major
    aug = np.empty((KAUG, 2 * b, n), dtype=ml_dtypes.bfloat16)
    for bi in range(b):
        aug[:, bi, :] = ax[bi].T.astype(ml_dtypes.bfloat16)
        aug[:, b + bi, :] = ay[bi].T.astype(ml_dtypes.bfloat16)
    return aug


_CACHE = {}


def _get_nc():
    if "nc" not in _CACHE:
        _CACHE["nc"] = build_nc(
            b_loc=B_FULL // N_CORES, n=N_FULL, c_in=C_FULL, num_devices=N_CORES
        )
    return _CACHE["nc"]


def make_in_maps(x: np.ndarray, y: np.ndarray):
    bl = B_FULL // N_CORES
    return [
        {"aug": _host_aug(x[i * bl:(i + 1) * bl], y[i * bl:(i + 1) * bl])}
        for i in range(N_CORES)
    ]


def kernel(x: np.ndarray, y: np.ndarray) -> np.ndarray:
    x = np.ascontiguousarray(np.asarray(x, dtype=np.float32))
    y = np.ascontiguousarray(np.asarray(y, dtype=np.float32))
    assert x.shape == (B_FULL, N_FULL, C_FULL), x.shape
    nc = _get_nc()
    in_maps = make_in_maps(x, y)
    res = run_bass_kernel_spmd(nc, in_maps, list(range(N_CORES)))
    total = sum(float(r["partial"].astype(np.float64).sum()) for r in res.results)
    loss = -total / float(B_FULL * N_FULL)
    return np.float32(loss)
